# revision 5
# baseline (speedup 1.0000x reference)
"""Trainium2 Bass kernel for nn_Critic (LSTM critic, B=512, T=100).

Strategy: data-parallel over batch (64 rows/core x 8 cores), fused
producer/consumer loop:
  - x-path (xz_t = x_t @ Wk + bl) is computed 2 steps at a time with the
    batch-of-2-steps packed into the stationary operand (full 128-wide PE
    array), results held in an SBUF ring (no DRAM round-trip). The bias add
    rides the PSUM->SBUF evacuation (DVE tensor_add with a replicated bl).
  - recurrent path: per step z = xz_t + h @ Wrk via a 4-way tile_position-
    packed identity inject (2 steps per 512-cycle pack) plus 2-way
    col-packed h@Wrk matmuls; gates/state in bf16; h transposed back to
    feature-major on the PE.
  - x-path work is interleaved between the recurrent matmuls of consecutive
    steps so the PE never idles while the gate chain (ACT/DVE) runs.
All matmuls bf16 (1 cycle/row); accumulation in PSUM f32.
"""
import numpy as np
import ml_dtypes

import concourse.bass as bass
import concourse.mybir as mybir
import concourse.tile as tile
from concourse import bacc
from concourse.bass_utils import run_bass_kernel_spmd
from concourse.masks import make_identity

F32 = mybir.dt.float32
F32R = mybir.dt.float32r
BF16 = mybir.dt.bfloat16
AF = mybir.ActivationFunctionType
ALU = mybir.AluOpType

NCORES = 8
BL = 64  # batch rows per core
LA = 2   # lookahead tiles for the x-path producer

_prog_cache = {}


def _elu(nc, pool, out_ap, psum_ap, bias_ap, P, R):
    """out = elu(psum + bias) = relu(x) + min(exp(x), 1) - 1."""
    ee = pool.tile([128, 128], BF16, tag="elu_e")
    rr = pool.tile([128, 128], BF16, tag="elu_r")
    nc.scalar.activation(ee[:P, :R], psum_ap, AF.Exp, bias=bias_ap)
    nc.scalar.activation(rr[:P, :R], psum_ap, AF.Relu, bias=bias_ap)
    nc.vector.scalar_tensor_tensor(
        ee[:P, :R], ee[:P, :R], 1.0, rr[:P, :R], ALU.min, ALU.add
    )
    nc.vector.tensor_scalar_add(out_ap, ee[:P, :R], -1.0)


def _build(T):
    TT = 2 * T - 1
    NROWT = (TT * BL + 127) // 128  # == T (last tile has 64 rows)

    nc = bacc.Bacc("TRN2", target_bir_lowering=False, num_devices=NCORES)

    # ---- I/O ----
    d_mot = nc.dram_tensor("mot", [BL, 64], F32, kind="ExternalInput")
    d_rob = nc.dram_tensor("rob", [BL, 128], F32, kind="ExternalInput")
    d_s1T = nc.dram_tensor("s1T", [TT, 128, BL], BF16, kind="ExternalInput")
    d_soT = nc.dram_tensor("soT", [TT, 128, BL], BF16, kind="ExternalInput")
    d_Wm = nc.dram_tensor("Wm", [64, 256], F32, kind="ExternalInput")
    d_bm = nc.dram_tensor("bm", [256, 1], F32, kind="ExternalInput")
    d_Wr = nc.dram_tensor("Wr", [128, 256], F32, kind="ExternalInput")
    d_br = nc.dram_tensor("br", [256, 1], F32, kind="ExternalInput")
    d_Wc = nc.dram_tensor("Wc", [512, 512], F32, kind="ExternalInput")
    d_bc = nc.dram_tensor("bc", [512, 1], F32, kind="ExternalInput")
    d_Wor = nc.dram_tensor("Wor", [128, 256], BF16, kind="ExternalInput")
    d_bor = nc.dram_tensor("bor", [256, 1], F32, kind="ExternalInput")
    d_Woi = nc.dram_tensor("Woi", [128, 256], BF16, kind="ExternalInput")
    d_boi = nc.dram_tensor("boi", [256, 1], F32, kind="ExternalInput")
    d_Wk = nc.dram_tensor("Wk", [640, 2048], BF16, kind="ExternalInput")
    d_Wrk = nc.dram_tensor("Wrk", [512, 2048], BF16, kind="ExternalInput")
    d_blf = nc.dram_tensor("blf", [128, 2048], F32, kind="ExternalInput")
    d_Wo = nc.dram_tensor("Wo", [128, 4], BF16, kind="ExternalInput")
    d_bo = nc.dram_tensor("bo", [64, 1], F32, kind="ExternalInput")
    d_tok = nc.dram_tensor("tok", [8, 8], F32, kind="ExternalInput")
    d_out = nc.dram_tensor("out", [BL, 1], F32, kind="ExternalOutput")
    d_tok_out = nc.dram_tensor("tok_out", [8, 8], F32, kind="ExternalOutput")

    with tile.TileContext(nc) as tc:
        # chain token passthrough (timing harness support; off critical path)
        nc.sync.dma_start(d_tok_out[:], d_tok[:])

        with tc.tile_pool(name="consts", bufs=1) as consts, \
             tc.tile_pool(name="wstage", bufs=2) as wstage, \
             tc.tile_pool(name="state", bufs=2) as state:

            # ---- identities ----
            identF = consts.tile([128, 128], F32, tag="identF")
            make_identity(nc, identF[:])
            identFB = consts.tile([128, 128], BF16, tag="identFB")
            nc.vector.tensor_copy(identFB[:], identF[:])

            # ---- weights (bf16 straight from DRAM) ----
            WkB = consts.tile([128, 5, 2048], BF16, tag="Wk")
            for k in range(5):
                nc.sync.dma_start(WkB[:, k, :], d_Wk[k * 128:(k + 1) * 128, :])
            WrkB = consts.tile([128, 4, 2048], BF16, tag="Wrk")
            for k in range(4):
                nc.sync.dma_start(WrkB[:, k, :], d_Wrk[k * 128:(k + 1) * 128, :])
            WorB = consts.tile([128, 256], BF16, tag="Wor")
            nc.sync.dma_start(WorB[:], d_Wor[:])
            WoiB = consts.tile([128, 256], BF16, tag="Woi")
            nc.sync.dma_start(WoiB[:], d_Woi[:])
            WoB = consts.tile([128, 4], BF16, tag="Wo")
            nc.sync.dma_start(WoB[:], d_Wo[:])
            blf = consts.tile([128, 2048], F32, tag="blf")
            nc.sync.dma_start(blf[:], d_blf[:])

            # ---- preamble weights (f32 -> f32r staging, one-time) ----
            def load_r(dram_ap, shape, tag, kslices=None):
                t = consts.tile(shape, F32R, tag=tag)
                if kslices is None:
                    st = wstage.tile(shape, F32, tag="wtmp_small")
                    nc.sync.dma_start(st[:], dram_ap)
                    nc.vector.tensor_copy(t[:], st[:])
                else:
                    nk, ncols = shape[1], shape[2]
                    for k in range(nk):
                        st = wstage.tile([128, 512], F32, tag="wtmp")
                        nc.sync.dma_start(
                            st[:, 0:ncols], dram_ap[k * 128:(k + 1) * 128, :]
                        )
                        nc.vector.tensor_copy(t[:, k, :], st[:, 0:ncols])
                return t

            WcR = load_r(d_Wc, [128, 4, 512], "Wc", kslices=True)
            WmR = load_r(d_Wm[:], [64, 256], "Wm")
            WrR = load_r(d_Wr[:], [128, 256], "Wr")

            # ---- per-partition biases (f32) ----
            def load_bias(dram, n_tiles, tag):
                t = consts.tile([128, n_tiles], F32, tag=tag)
                for m in range(n_tiles):
                    nc.sync.dma_start(t[:, m:m + 1], dram[m * 128:(m + 1) * 128, :])
                return t

            bm_t = load_bias(d_bm, 2, "bm")
            br_t = load_bias(d_br, 2, "br")
            bc_t = load_bias(d_bc, 4, "bc")
            bor_t = load_bias(d_bor, 2, "bor")
            boi_t = load_bias(d_boi, 2, "boi")
            bo_t = consts.tile([64, 1], F32, tag="bo")
            nc.sync.dma_start(bo_t[:], d_bo[:])

            # ================= preamble: h0 = c0 = state =================
            with tc.tile_pool(name="pre", bufs=2) as pre, \
                 tc.tile_pool(name="pre_ps", bufs=2, space="PSUM") as pre_ps, \
                 tc.tile_pool(name="pre_tp", bufs=2, space="PSUM") as pre_tp:

                def _elu_f32(out_ap, psum_ap, bias_ap, P, R):
                    ee = pre.tile([128, 128], F32, tag="pelu_e")
                    rr = pre.tile([128, 128], F32, tag="pelu_r")
                    nc.scalar.activation(ee[:P, :R], psum_ap, AF.Exp, bias=bias_ap)
                    nc.scalar.activation(rr[:P, :R], psum_ap, AF.Relu, bias=bias_ap)
                    nc.vector.scalar_tensor_tensor(
                        ee[:P, :R], ee[:P, :R], 1.0, rr[:P, :R], ALU.min, ALU.add
                    )
                    nc.vector.tensor_scalar_add(out_ap, ee[:P, :R], -1.0)

                s_mot = pre.tile([64, 64], F32, tag="s_mot")
                nc.sync.dma_start(s_mot[:], d_mot[:])
                s_rob = pre.tile([64, 128], F32, tag="s_rob")
                nc.sync.dma_start(s_rob[:], d_rob[:])

                tp0 = pre_tp.tile([128, 256], F32, tag="tp")
                nc.tensor.transpose(tp0[0:64, 0:64], s_mot[:], identF[0:64, 0:64])
                nc.tensor.transpose(tp0[:, 64:128], s_rob[:], identF[0:64, 0:64])
                motT = pre.tile([64, 64], F32R, tag="motT")
                nc.vector.tensor_copy(motT[:], tp0[0:64, 0:64])
                robT = pre.tile([128, 64], F32R, tag="robT")
                nc.vector.tensor_copy(robT[:], tp0[:, 64:128])

                # ms / rs  (feature-major [256, 64] as [128, 2*64])
                msT = pre.tile([128, 128], F32R, tag="msT")
                rsT = pre.tile([128, 128], F32R, tag="rsT")
                ps_m = pre_ps.tile([128, 128], F32, tag="small")
                for m in range(2):
                    nc.tensor.matmul(ps_m[:, m * 64:(m + 1) * 64],
                                     WmR[0:64, m * 128:(m + 1) * 128], motT[:],
                                     start=True, stop=True)
                for m in range(2):
                    _elu_f32(msT[:, m * 64:(m + 1) * 64],
                             ps_m[:, m * 64:(m + 1) * 64], bm_t[:, m:m + 1], 128, 64)
                ps_r = pre_ps.tile([128, 128], F32, tag="small")
                for m in range(2):
                    nc.tensor.matmul(ps_r[:, m * 64:(m + 1) * 64],
                                     WrR[:, m * 128:(m + 1) * 128], robT[:],
                                     start=True, stop=True)
                for m in range(2):
                    _elu_f32(rsT[:, m * 64:(m + 1) * 64],
                             ps_r[:, m * 64:(m + 1) * 64], br_t[:, m:m + 1], 128, 64)

                # state = elu([ms, rs] @ Wc + bc) -> st01/st23 (hT layout) + c0
                st01 = state.tile([128, 128], BF16, tag="hT01")
                st23 = state.tile([128, 128], BF16, tag="hT23")
                stF = pre.tile([128, 256], F32, tag="stF")
                ps_c = pre_ps.tile([128, 256], F32, tag="small")
                for G in range(4):
                    reg = ps_c[:, G * 64:(G + 1) * 64]
                    for k in range(4):
                        rhs = msT[:, (k % 2) * 64:(k % 2) * 64 + 64] if k < 2 \
                            else rsT[:, (k % 2) * 64:(k % 2) * 64 + 64]
                        nc.tensor.matmul(reg, WcR[:, k, G * 128:(G + 1) * 128],
                                         rhs, start=(k == 0), stop=(k == 3))
                for G in range(4):
                    _elu_f32(stF[:, G * 64:(G + 1) * 64],
                             ps_c[:, G * 64:(G + 1) * 64], bc_t[:, G:G + 1], 128, 64)
                nc.vector.tensor_copy(st01[:], stF[:, 0:128])
                nc.vector.tensor_copy(st23[:], stF[:, 128:256])

                c0 = state.tile([128, 256], BF16, tag="c")
                tp1 = pre_tp.tile([128, 256], F32, tag="tp")
                nc.tensor.transpose(tp1[:, 0:128], stF[:, 0:128], identF[:])
                nc.tensor.transpose(tp1[:, 128:256], stF[:, 128:256], identF[:])
                nc.vector.tensor_copy(c0[:, 0:128], tp1[:, 0:128])
                nc.vector.tensor_copy(c0[:, 128:256], tp1[:, 128:256])

            # ================= fused main loop =================
            with tc.tile_pool(name="ring", bufs=LA + 1) as ring, \
                 tc.tile_pool(name="xin", bufs=2) as xin, \
                 tc.tile_pool(name="i23", bufs=2) as i23, \
                 tc.tile_pool(name="pb", bufs=2) as pb, \
                 tc.tile_pool(name="zx", bufs=2, space="PSUM") as zx, \
                 tc.tile_pool(name="zw", bufs=1, space="PSUM") as zw, \
                 tc.tile_pool(name="zp", bufs=2, space="PSUM") as zpp, \
                 tc.tile_pool(name="tp", bufs=1, space="PSUM") as tpp:

                slots = {}

                def stage_in(R):
                    """Load + transform inputs of tile R -> (s1t, i2T, i3T)."""
                    rows = 128 if R < NROWT - 1 else TT * BL - 128 * (NROWT - 1)
                    nst = rows // 64
                    s1t = xin.tile([128, 2, 64], BF16, tag="s1t")
                    nc.sync.dma_start(
                        s1t[:, 0:nst, :],
                        d_s1T[2 * R:2 * R + nst].rearrange("t f b -> f t b"))
                    sot = xin.tile([128, 2, 64], BF16, tag="sot")
                    nc.sync.dma_start(
                        sot[:, 0:nst, :],
                        d_soT[2 * R:2 * R + nst].rearrange("t f b -> f t b"))

                    # inp2 = elu(osc_half @ Wor + bor)   (feature-major)
                    i2T = i23.tile([128, 256], BF16, tag="i2T")
                    ps2 = zw.tile([128, 256], F32, tag="zw")
                    for m in range(2):
                        nc.tensor.matmul(ps2[:, m * 128:m * 128 + rows],
                                         WorB[:, m * 128:(m + 1) * 128],
                                         sot[:, 0:nst, :], start=True, stop=True)
                    for m in range(2):
                        _elu(nc, pb, i2T[:, m * 128:m * 128 + rows],
                             ps2[:, m * 128:m * 128 + rows], bor_t[:, m:m + 1],
                             128, rows)
                    # inp3 = elu(inp2[:, 128:256] @ Woi + boi)
                    i3T = i23.tile([128, 256], BF16, tag="i3T")
                    ps3 = zw.tile([128, 256], F32, tag="zw")
                    for m in range(2):
                        nc.tensor.matmul(ps3[:, m * 128:m * 128 + rows],
                                         WoiB[:, m * 128:(m + 1) * 128],
                                         i2T[:, 128:128 + rows], start=True, stop=True)
                    for m in range(2):
                        _elu(nc, pb, i3T[:, m * 128:m * 128 + rows],
                             ps3[:, m * 128:m * 128 + rows], boi_t[:, m:m + 1],
                             128, rows)
                    return rows, s1t, i2T, i3T

                def xz_half(R, half):
                    """x @ Wk for one 1024-col half; evac (+bl) into ring slot."""
                    rows, s1t, i2T, i3T = stage_ctx[R]
                    if half == 0:
                        slot = ring.tile([128, 2048], BF16, tag="ring")
                        slots[R] = slot
                    slot = slots[R]
                    lhs = [s1t[:, 0:rows // 64, :], i2T[:, 0:rows],
                           i2T[:, 128:128 + rows], i3T[:, 0:rows],
                           i3T[:, 128:128 + rows]]
                    psz = zx.tile([128, 512], F32, tag="zx")
                    psz2 = zx.tile([128, 512], F32, tag="zx")
                    for k in range(5):
                        nc.tensor.matmul(
                            psz[0:rows, :], lhs[k],
                            WkB[:, k, (2 * half) * 512:(2 * half + 1) * 512],
                            start=(k == 0), stop=(k == 4))
                        nc.tensor.matmul(
                            psz2[0:rows, :], lhs[k],
                            WkB[:, k, (2 * half + 1) * 512:(2 * half + 2) * 512],
                            start=(k == 0), stop=(k == 4))
                    c0_ = (2 * half) * 512
                    nc.vector.tensor_add(slot[0:rows, c0_:c0_ + 512],
                                         psz[0:rows, :], blf[0:rows, c0_:c0_ + 512])
                    nc.vector.tensor_add(slot[0:rows, c0_ + 512:c0_ + 1024],
                                         psz2[0:rows, :],
                                         blf[0:rows, c0_ + 512:c0_ + 1024])

                def emit_inject(pair):
                    """Inject xz (+bl) for steps 2*pair, 2*pair+1 into fresh PSUM."""
                    slot = slots.pop(pair)
                    t1_valid = 2 * pair + 1 < TT
                    zp0 = zpp.tile([128, 1024], F32, tag="zp", name="zp0")
                    zp1 = (zpp.tile([128, 1024], F32, tag="zp", name="zp1")
                           if t1_valid else None)
                    for b in range(2):
                        nc.tensor.matmul(zp0[0:64, b * 512:(b + 1) * 512],
                                         identFB[0:64, 0:64],
                                         slot[0:64, (2 * b) * 512:(2 * b + 1) * 512],
                                         start=True, stop=False, tile_position=(0, 0))
                        nc.tensor.matmul(zp0[64:128, b * 512:(b + 1) * 512],
                                         identFB[0:64, 0:64],
                                         slot[0:64, (2 * b + 1) * 512:(2 * b + 2) * 512],
                                         start=True, stop=False, tile_position=(0, 64))
                        if t1_valid:
                            nc.tensor.matmul(zp1[0:64, b * 512:(b + 1) * 512],
                                             identFB[64:128, 64:128],
                                             slot[64:128, (2 * b) * 512:(2 * b + 1) * 512],
                                             start=True, stop=False,
                                             tile_position=(64, 0))
                            nc.tensor.matmul(zp1[64:128, b * 512:(b + 1) * 512],
                                             identFB[64:128, 64:128],
                                             slot[64:128, (2 * b + 1) * 512:(2 * b + 2) * 512],
                                             start=True, stop=False,
                                             tile_position=(64, 64))
                    return zp0, zp1

                def g_mms(zp, hT01, hT23):
                    for G in range(4):
                        lhsT = (hT01 if G < 2 else hT23)[:, (G % 2) * 64:(G % 2) * 64 + 64]
                        for b in range(2):
                            nc.tensor.matmul(zp[0:64, b * 512:(b + 1) * 512], lhsT,
                                             WrkB[:, G, (2 * b) * 512:(2 * b + 1) * 512],
                                             start=False, stop=(G == 3),
                                             tile_position=(0, 0))
                            nc.tensor.matmul(zp[64:128, b * 512:(b + 1) * 512], lhsT,
                                             WrkB[:, G, (2 * b + 1) * 512:(2 * b + 2) * 512],
                                             start=False, stop=(G == 3),
                                             tile_position=(0, 64))

                def gates(zp, c_prev):
                    """z -> (h tiles hb0/hb1, c_new); all bf16."""
                    c_new = state.tile([128, 256], BF16, tag="c")
                    hbs = []
                    for b in range(2):
                        sg = pb.tile([128, 384], BF16, tag="sg")
                        nc.scalar.activation(sg[:], zp[:, b * 512:b * 512 + 384],
                                             AF.Sigmoid)
                        tg = pb.tile([128, 128], BF16, tag="tg")
                        nc.scalar.activation(tg[:], zp[:, b * 512 + 384:b * 512 + 512],
                                             AF.Tanh)
                        t1 = pb.tile([128, 128], BF16, tag="t1")
                        nc.vector.tensor_mul(t1[:], sg[:, 0:128], tg[:])
                        t2 = pb.tile([128, 128], BF16, tag="t2")
                        nc.vector.tensor_mul(t2[:], sg[:, 128:256],
                                             c_prev[:, b * 128:(b + 1) * 128])
                        cs = c_new[:, b * 128:(b + 1) * 128]
                        nc.vector.tensor_add(cs, t1[:], t2[:])
                        tcn = pb.tile([128, 128], BF16, tag="tc")
                        nc.scalar.activation(tcn[:], cs, AF.Tanh)
                        hb = pb.tile([128, 128], BF16, tag="hb")
                        nc.vector.tensor_mul(hb[:], sg[:, 256:384], tcn[:])
                        hbs.append(hb)
                    return hbs, c_new

                def transp_h(hbs):
                    hT01_n = state.tile([128, 128], BF16, tag="hT01")
                    hT23_n = state.tile([128, 128], BF16, tag="hT23")
                    tpb = tpp.tile([128, 256], BF16, tag="tp")
                    for b in range(2):
                        nc.tensor.transpose(tpb[:, b * 128:(b + 1) * 128],
                                            hbs[b][:], identFB[:])
                        dst = hT01_n if b == 0 else hT23_n
                        nc.scalar.copy(dst[:], tpb[:, b * 128:(b + 1) * 128])
                    return hT01_n, hT23_n

                # ---- prologue: fill the x-path pipeline ----
                # stage_in runs 2 tiles ahead of the consumer, xz MMs 1 tile
                # ahead, so the elu chain (ACT/DVE) of tile R+2 overlaps the
                # xz matmuls of tile R+1 and the recurrent work of pair R.
                stage_ctx = {}
                stage_ctx[0] = stage_in(0)
                if NROWT > 1:
                    stage_ctx[1] = stage_in(1)
                xz_half(0, 0)
                xz_half(0, 1)
                zp_cur = emit_inject(0)

                hT01, hT23, c_prev = st01, st23, c0
                NPAIR = (TT + 1) // 2
                for pair in range(NPAIR):
                    t0 = 2 * pair
                    t1_valid = t0 + 1 < TT
                    zp0, zp1 = zp_cur
                    Rs = pair + 2   # tile entering stage_in
                    Rx = pair + 1   # tile whose xz matmuls run this pair

                    # step t0
                    g_mms(zp0, hT01, hT23)
                    hbs0, c_new0 = gates(zp0, c_prev)
                    if Rs < NROWT:
                        stage_ctx[Rs] = stage_in(Rs)
                    if Rx < NROWT:
                        xz_half(Rx, 0)
                    hT01, hT23 = transp_h(hbs0)
                    c_prev = c_new0

                    if t1_valid:
                        # step t1
                        g_mms(zp1, hT01, hT23)
                        hbs1, c_new1 = gates(zp1, c_prev)
                        if Rx < NROWT:
                            xz_half(Rx, 1)
                        if pair + 1 < NPAIR:
                            zp_cur = emit_inject(pair + 1)
                        hT01, hT23 = transp_h(hbs1)
                        c_prev = c_new1
                    elif pair + 1 < NPAIR:
                        zp_cur = emit_inject(pair + 1)

                # ---- output: elu(h @ Wo + bo) ----
                ps_o = zpp.tile([64, 512], F32, tag="zp")
                for G in range(4):
                    lhsT = (hT01 if G < 2 else hT23)[:, (G % 2) * 64:(G % 2) * 64 + 64]
                    nc.tensor.matmul(ps_o[:, 0:1], lhsT, WoB[:, G:G + 1],
                                     start=(G == 0), stop=(G == 3))
                out_sb = pb.tile([64, 1], F32, tag="out_sb")
                ee = pb.tile([64, 1], F32, tag="oee")
                rr = pb.tile([64, 1], F32, tag="orr")
                nc.scalar.activation(ee[:], ps_o[:, 0:1], AF.Exp, bias=bo_t[:])
                nc.scalar.activation(rr[:], ps_o[:, 0:1], AF.Relu, bias=bo_t[:])
                nc.vector.scalar_tensor_tensor(ee[:], ee[:], 1.0, rr[:],
                                               ALU.min, ALU.add)
                nc.vector.tensor_scalar_add(out_sb[:], ee[:], -1.0)
                nc.sync.dma_start(d_out[:], out_sb[:])

    nc.compile()
    return nc


def _gate_perm():
    """Column permutation: [i|f|g|o] blocks of 512 -> per-128-unit-group [i f o g]."""
    perm = []
    for G in range(4):
        for gate in (0, 1, 3, 2):  # i, f, o, g
            perm.extend(range(gate * 512 + G * 128, gate * 512 + (G + 1) * 128))
    return np.array(perm)


def _prepare(inputs):
    motion_state = np.asarray(inputs["motion_state"], np.float32)
    robot_state = np.asarray(inputs["robot_state"], np.float32)
    action = np.asarray(inputs["action"], np.float32)
    osc = np.asarray(inputs["osc"], np.float32)
    history = np.asarray(inputs["history"], np.float32)
    history_osc = np.asarray(inputs["history_osc"], np.float32)

    B, T = action.shape[0], action.shape[1]
    assert B == NCORES * BL
    TT = 2 * T - 1
    BF = ml_dtypes.bfloat16

    perm = _gate_perm()
    Wk_p = np.ascontiguousarray(
        np.asarray(inputs["Wk"], np.float32)[:, perm]).astype(BF)
    Wrk_p = np.ascontiguousarray(
        np.asarray(inputs["Wrk"], np.float32)[:, perm]).astype(BF)
    bl_p = np.asarray(inputs["bl"], np.float32)[perm].reshape(1, 2048)
    blf = np.ascontiguousarray(np.broadcast_to(bl_p, (128, 2048)).astype(np.float32))
    Wo = np.asarray(inputs["Wo"], np.float32)  # [512, 1]
    Wo_t = np.ascontiguousarray(Wo.reshape(4, 128).T).astype(BF)  # [128, 4]
    bo = np.asarray(inputs["bo"], np.float32)
    bo_t = np.full((64, 1), float(bo[0]), np.float32)

    # streams: warmup over last T-1 history frames, then action/osc,
    # pre-transposed to feature-major [TT, 128, B] bf16
    seq1 = np.concatenate([history[:, 1:], action], axis=1)          # [B, TT, 128]
    seqo = np.concatenate([history_osc[:, 1:, 0:128], osc[:, :, 0:128]], axis=1)
    seq1_T = np.ascontiguousarray(seq1.transpose(1, 2, 0)).astype(BF)  # [TT,128,B]
    seqo_T = np.ascontiguousarray(seqo.transpose(1, 2, 0)).astype(BF)

    tok = np.zeros((8, 8), np.float32)
    shared = {
        "Wm": np.asarray(inputs["Wm"], np.float32),
        "bm": np.asarray(inputs["bm"], np.float32).reshape(256, 1),
        "Wr": np.asarray(inputs["Wr"], np.float32),
        "br": np.asarray(inputs["br"], np.float32).reshape(256, 1),
        "Wc": np.asarray(inputs["Wc"], np.float32),
        "bc": np.asarray(inputs["bc"], np.float32).reshape(512, 1),
        "Wor": np.asarray(inputs["Wor"], np.float32).astype(BF),
        "bor": np.asarray(inputs["bor"], np.float32).reshape(256, 1),
        "Woi": np.asarray(inputs["Woi"], np.float32).astype(BF),
        "boi": np.asarray(inputs["boi"], np.float32).reshape(256, 1),
        "Wk": Wk_p, "Wrk": Wrk_p, "blf": blf, "Wo": Wo_t, "bo": bo_t,
        "tok": tok,
    }

    in_maps = []
    for c in range(NCORES):
        sl = slice(c * BL, (c + 1) * BL)
        m = dict(shared)
        m["mot"] = np.ascontiguousarray(motion_state[sl])
        m["rob"] = np.ascontiguousarray(robot_state[sl])
        m["s1T"] = np.ascontiguousarray(seq1_T[:, :, sl])
        m["soT"] = np.ascontiguousarray(seqo_T[:, :, sl])
        in_maps.append(m)

    return in_maps, T


def kernel(**inputs):
    in_maps, T = _prepare(inputs)
    if T not in _prog_cache:
        _prog_cache[T] = _build(T)
    nc = _prog_cache[T]

    res = run_bass_kernel_spmd(nc, in_maps, core_ids=list(range(NCORES)))
    out = np.concatenate([res.results[c]["out"] for c in range(NCORES)], axis=0)
    return out.astype(np.float32)
